# revision 60
# baseline (speedup 1.0000x reference)
"""Multi-head attention (dense_transformer) Trainium2 Bass kernel, v2.

Problem: x[8, 512, 32, 32]; per-batch 1x1-conv QKV projections, 8-head
attention over N=H*W=1024 positions (head_dim 64), output projection,
residual. Sharding: data-parallel over batch B=8 across the 8 cores -
one batch element per core, no collectives.

v2 strategy (vs the bf16 baseline): run every matmul in fp8e4 with the
DoubleRow perf mode.  A DoubleRow matmul takes lhsT [K,2,M] / rhs
[K,2,N] and computes sum_g W[:,g].T @ X[:,g] - two 128-deep contraction
steps in one instruction at 0.5 cycles per output row, i.e. 4x the
bf16 matmul throughput per unit of contracted work.  That collapses the
PE time from ~82us to ~30us and shifts the bottleneck to the softmax
exp (~65k free-elems), which is split across three engines:

  - ScalarE (Act): true exp via the activation LUT.
  - VectorE / GpSimd: Schraudolph-style fast exp - one fused
    tensor_scalar (x*log2e + bias) written as int8 IS the fp8e4 bit
    pattern of exp(x/8).  (e4m3 bits b ~= 8*(log2(v)+7), so
    b = s*log2(e) + 56 up to a mantissa correction.)

Other tricks:
  - All five "c = 512" contractions map c -> (ctpair, group, partition)
    so DoubleRow pairs come straight from SBUF layout; Wq/Wk columns are
    host-permuted so each head's Q/K lands as [32, 2, N] (d-halves in
    the group dim) on its own 32-partition band, making S^T a single
    DoubleRow matmul per output tile.
  - V is projected transposed (VT[j, head, d]) with stationary columns
    64..127 set to 1.0, so the AV matmul leaves rows 64..127 of PSUM
    holding the softmax denominator replicated 64x - normalization is
    one partition-offset tensor_tensor divide, no broadcasts.
  - Output projection accumulates Wo.T O (fp8 DoubleRow), + bo (K=1
    DoubleRow bias matmul), + x (bf16 identity matmul residual) in one
    PSUM group; the only epilogue is a PSUM->SBUF copy before DMA out.
"""

import sys

if "/opt/trn_rl_repo" not in sys.path:
    sys.path.insert(0, "/opt/trn_rl_repo")

import numpy as np
import ml_dtypes

import concourse.bass as bass
import concourse.mybir as mybir
from concourse.tile import TileContext

DIM = 512
NH = 8
HD = 64
N = 1024
P = 128
F32 = mybir.dt.float32
BF16 = mybir.dt.bfloat16
F8 = mybir.dt.float8e4
I8 = mybir.dt.int8
AOP = mybir.AluOpType
EXP = mybir.ActivationFunctionType.Exp
DR = mybir.MatmulPerfMode.DoubleRow

# Schraudolph fast-exp constants for e4m3 bits: for logits s (pre 1/8
# scale), bits = s*log2(e) + 7*8 - 0.344 (mantissa-linearity centering)
# + 0.5 (CoreSim truncates on f32->i8 write; HW rounding only shifts
# the softmax by a uniform factor that the normalization cancels).
EXP_C1 = 1.4426950408889634
EXP_C2 = 56.0 - 0.344 + 0.5


class FixedTileContext(TileContext):
    """Works around a walrus/bass snapshot mismatch: this walrus build
    accepts only one sync-wait command per instruction, but Tile's wait
    assigner happily attaches several. After scheduling, excess waits on
    any instruction are peeled off onto same-engine NOPs inserted right
    before it (same blocking semantics: the engine executes in order)."""

    MAX_WAITS = 1
    MAX_WAITS_DATA = 1
    _wsplit_ctr = 0

    def _split_sync_waits(self):
        seq_only = mybir.SEQUENCER_ONLY_OPCODES
        for fn in self.nc.m.functions:
            for blk in fn.blocks:
                insts = list(blk.instructions)
                out = []
                for inst in insts:
                    si = inst.sync_info
                    limit = (
                        self.MAX_WAITS
                        if inst.opcode in seq_only
                        else self.MAX_WAITS_DATA
                    )
                    if si is not None and len(si.on_wait) > limit:
                        waits = list(si.on_wait)
                        movers = waits[:-limit]
                        keep = waits[-limit:]
                        del si.on_wait[:]
                        for w in keep:
                            si.on_wait.append(w)
                        for w in movers:
                            FixedTileContext._wsplit_ctr += 1
                            nop = mybir.InstNoOp(
                                name=f"wsplit-{FixedTileContext._wsplit_ctr}",
                                ins=[],
                                outs=[],
                            )
                            nop.engine = inst.engine
                            nop.sync_info = mybir.SyncInfo(on_wait=[w], on_update=[])
                            out.append(nop)
                    out.append(inst)
                if len(out) != len(insts):
                    del blk.instructions[:]
                    for i in out:
                        blk.add_instruction(i)

    split_on_exit = True

    def __exit__(self, *exc):
        ret = super().__exit__(*exc)
        if exc[0] is None and self.split_on_exit:
            self._split_sync_waits()
        return ret


def build_nc(split_waits=True):
    nc = bass.Bass()

    x8d = nc.dram_tensor("x8", [2, P, 2, N], F8, kind="ExternalInput")
    x16d = nc.dram_tensor("x16", [4, P, N], BF16, kind="ExternalInput")
    wqkd = nc.dram_tensor("wqk8", [2, P, 2, 2, DIM], F8, kind="ExternalInput")
    wvod = nc.dram_tensor("wvo8", [2, P, 2, 2, DIM], F8, kind="ExternalInput")
    bqkd = nc.dram_tensor("bqk", [P, 8], F32, kind="ExternalInput")
    bvd = nc.dram_tensor("bv", [DIM], F32, kind="ExternalInput")
    bo8d = nc.dram_tensor("bo8", [1, 2, DIM], F8, kind="ExternalInput")
    onesd = nc.dram_tensor("ones8", [N], F8, kind="ExternalInput")
    zerosd = nc.dram_tensor("zeros8", [2048], F8, kind="ExternalInput")
    identd = nc.dram_tensor("ident16", [P, P], BF16, kind="ExternalInput")
    outd = nc.dram_tensor("out", [4, P, N], BF16, kind="ExternalOutput")

    FixedTileContext.split_on_exit = split_waits
    with FixedTileContext(nc) as tc:
        with tc.tile_pool(name="persist", bufs=1) as persist, tc.tile_pool(
            name="otile", bufs=4
        ) as otile, tc.tile_pool(name="rec", bufs=2) as rec_pool:
            # ---------------- loads ----------------
            # Startup-critical DMAs are split into halves and spread over
            # the SP and Act HWDGE queues so the first Q-projection group
            # (x8 n-half 0 + the two Wq halves) lands as early as possible.
            x8sb = [
                persist.tile([P, 2, N], F8, tag=f"x8_{cp}", name=f"x8_{cp}")
                for cp in range(2)
            ]
            wqk = [
                persist.tile([P, 2, 2, DIM], F8, tag=f"wqk_{cp}", name=f"wqk_{cp}")
                for cp in range(2)
            ]
            for cp in range(2):
                nc.sync.dma_start(
                    out=x8sb[cp][:, :, 0:512], in_=x8d[cp][:, :, 0:512]
                )
                nc.scalar.dma_start(
                    out=wqk[cp][:, :, 0, :], in_=wqkd[cp][:, :, 0, :]
                )
            bqk_sb = persist.tile([P, 8], F32, tag="bqk", name="bqk")
            nc.sync.dma_start(out=bqk_sb, in_=bqkd[:, :])
            # K weight halves before the x8 i-halves: the shared DMA
            # device serializes transfers in issue order, and the first
            # K-projection matmuls need these ~1.5us before the x8
            # second halves are touched.
            for cp in range(2):
                nc.scalar.dma_start(
                    out=wqk[cp][:, :, 1, :], in_=wqkd[cp][:, :, 1, :]
                )
            for cp in range(2):
                nc.sync.dma_start(
                    out=x8sb[cp][:, :, 512:N], in_=x8d[cp][:, :, 512:N]
                )
            wvo = []
            for cp in range(2):
                t = persist.tile(
                    [P, 2, 2, DIM], F8, tag=f"wvo_{cp}", name=f"wvo_{cp}"
                )
                nc.sync.dma_start(out=t, in_=wvod[cp])
                wvo.append(t)
            # bv broadcast across partitions and the head-group dim via
            # zero strides on the DRAM side.
            bvB = persist.tile([P, 2, NH, HD], F32, tag="bvB", name="bvB")
            bv_ap = bvd[:]
            nc.scalar.dma_start(
                out=bvB,
                in_=bass.AP(
                    tensor=bv_ap.tensor,
                    offset=0,
                    ap=[[0, P], [0, 2], [1, DIM]],
                ),
            )

            # VT tiles: [j-partition, jt-group, head, 128] - V sits in
            # columns 64*(h%2)..+64, the other half is zero, so an M=128
            # matmul at column position 0 (the only legal one) lands the
            # head's raw O on partitions 64*(h%2)..+64 of the pair tile.
            VT = [
                persist.tile([P, 2, NH, P], F8, tag=f"vt_{jp}", name=f"vt_{jp}")
                for jp in range(4)
            ]
            ones_ap = onesd[:]
            zeros_ap = zerosd[:]
            for jp in range(4):
                nc.sync.dma_start(
                    out=VT[jp],
                    in_=bass.AP(
                        tensor=zeros_ap.tensor, offset=0,
                        ap=[[0, P], [1, 2048]],
                    ),
                )
            # denominator stationary operands: ones in the parity half,
            # zeros in the other
            onesAV = persist.tile([P, 2, 2, P], F8, tag="onesAV", name="onesAV")
            nc.sync.dma_start(
                out=onesAV,
                in_=bass.AP(
                    tensor=zeros_ap.tensor, offset=0, ap=[[0, P], [1, 512]]
                ),
            )
            for e in range(2):
                nc.sync.dma_start(
                    out=onesAV[:, :, e, e * HD : (e + 1) * HD],
                    in_=bass.AP(
                        tensor=ones_ap.tensor, offset=0,
                        ap=[[0, P], [1, 2 * HD]],
                    ),
                )
            # phase-3 inputs (SP queue keeps filling while PE works)
            x16sb = []
            for ot in range(4):
                t = persist.tile([P, N], BF16, tag=f"x16_{ot}", name=f"x16_{ot}")
                nc.sync.dma_start(out=t, in_=x16d[ot])
                x16sb.append(t)
            ident16 = persist.tile([P, P], BF16, tag="ident", name="ident")
            nc.sync.dma_start(out=ident16, in_=identd[:, :])
            bo8sb = persist.tile([1, 2, DIM], F8, tag="bo8", name="bo8")
            nc.sync.dma_start(out=bo8sb, in_=bo8d[:, :, :])
            ones8 = persist.tile([1, 2, DIM], F8, tag="ones8", name="ones8")
            nc.sync.dma_start(
                out=ones8,
                in_=bass.AP(tensor=ones_ap.tensor, offset=0, ap=[[0, 1], [1, N]]),
            )

            # persistent attention state
            Qs = [
                persist.tile([P, 2, N], F8, tag=f"qs_{i}", name=f"qs_{i}")
                for i in range(2)
            ]
            Ks = [
                persist.tile([P, 2, N], F8, tag=f"ks_{i}", name=f"ks_{i}")
                for i in range(2)
            ]
            # PE tile row positions only allow 0/32/64 - heads 3 and 7
            # (whose bands sit at partitions 96..127) are DMA-remapped
            # into spare tiles at rows 0:32 / 32:64.
            QsX = persist.tile([P, 2, N], F8, tag="qsx", name="qsx")
            KsX = persist.tile([P, 2, N], F8, tag="ksx", name="ksx")
            P8 = [
                [
                    persist.tile(
                        [P, 2, N], F8, tag=f"p8_{h}_{jp}", name=f"p8_{h}_{jp}"
                    )
                    for jp in range(4)
                ]
                for h in range(NH)
            ]
            O8 = [
                persist.tile([P, 2, N], F8, tag=f"o8_{cp}", name=f"o8_{cp}")
                for cp in range(2)
            ]

            # ------------- Q/K/V projections -------------
            # All phases share one PSUM pool structure (psA: 2x [128,1024]
            # / psB, psO: 2x [128,512]) so no pool-close barrier ever
            # stalls the pipeline between phases.
            ACT_JTS = ((0, 1, 2, 4, 5, 6), (0, 1, 2, 4, 6))

            with tc.tile_pool(
                name="psA", bufs=2, space="PSUM"
            ) as psA_pool, tc.tile_pool(
                name="psB", bufs=4, space="PSUM"
            ) as psB_pool:
                psO_pool = psB_pool

                def qk_proj_half(qk, ot, nh2, pool):
                    ps = pool.tile([P, 512], F32, tag=pool.name, name=f"pp{qk}{ot}{nh2}")
                    for cp in range(2):
                        nc.tensor.matmul(
                            ps,
                            lhsT=wqk[cp][:, :, qk, ot * P : (ot + 1) * P],
                            rhs=x8sb[cp][:, :, nh2 * 512 : (nh2 + 1) * 512],
                            start=(cp == 0),
                            stop=(cp == 1),
                            perf_mode=DR,
                        )
                    return ps

                def qk_epi_half(qk, ot, nh2, ps, eng):
                    tgt = (Qs if qk == 0 else Ks)[ot // 2][
                        :, ot % 2, nh2 * 512 : (nh2 + 1) * 512
                    ]
                    bias = bqk_sb[:, 4 * qk + ot : 4 * qk + ot + 1]
                    if eng is nc.scalar:
                        nc.scalar.activation(
                            tgt, ps, mybir.ActivationFunctionType.Identity, bias=bias
                        )
                    else:
                        eng.tensor_scalar(tgt, ps, bias, None, op0=AOP.add)

                def v_proj_half(vt, g, pool):
                    ps = pool.tile([P, 512], F32, tag=pool.name, name=f"ppv{vt}{g}")
                    jt = 2 * vt + g
                    for cp in range(2):
                        nc.tensor.matmul(
                            ps,
                            lhsT=x8sb[cp][:, :, jt * P : (jt + 1) * P],
                            rhs=wvo[cp][:, :, 0, :],
                            start=(cp == 0),
                            stop=(cp == 1),
                            perf_mode=DR,
                        )
                    return ps

                def v_epi_half(vt, g, ps, eng):
                    vte = VT[vt].rearrange("p g (h2 e) d -> p g h2 e d", e=2)
                    psr = ps.rearrange("p (h2 e d) -> p h2 e d", h2=4, e=2)
                    bvr = bvB.rearrange("p g (h2 e) d -> p g h2 e d", e=2)
                    for e in range(2):
                        eng.tensor_tensor(
                            vte[:, g, :, e, e * HD : (e + 1) * HD],
                            psr[:, :, e, :],
                            bvr[:, g, :, e, :],
                            AOP.add,
                        )

                # Q/K columns 0..1 first (they gate the first head's S);
                # epilogue halves spread over all three engines.  V after.
                for nh2 in range(2):
                    for ot in range(2):
                        for qk in range(2):
                            ps = qk_proj_half(qk, ot, nh2, psB_pool)
                            qk_epi_half(
                                qk, ot, nh2, ps,
                                nc.scalar if qk == 0 else nc.vector,
                            )
                # remap head 3's band to a legal PE row position (head
                # 7's follows once the deferred ot2/3 projections land)
                nc.sync.dma_start(out=QsX[0:32, :, :], in_=Qs[0][96:P, :, :])
                nc.sync.dma_start(out=KsX[0:32, :, :], in_=Ks[0][96:P, :, :])

                V_ENG = [nc.vector] * 8

                def v_proj_all():
                    for vt in range(4):
                        for g in range(2):
                            ps = v_proj_half(vt, g, psB_pool)
                            v_epi_half(vt, g, ps, V_ENG[2 * vt + g])

                # ------------- attention heads -------------
                def emit_s(h, jts, flip):
                    if h % 4 == 3:
                        Qt, Kt = QsX, KsX
                        p0 = 32 * (h // 4)
                    else:
                        Qt, Kt = Qs[h // 4], Ks[h // 4]
                        p0 = 32 * (h % 4)

                    def smm(out_ap, jt, ih):
                        nc.tensor.matmul(
                            out_ap,
                            lhsT=Kt[p0 : p0 + 32, :, jt * P : (jt + 1) * P],
                            rhs=Qt[p0 : p0 + 32, :, ih * 512 : (ih + 1) * 512],
                            start=True,
                            stop=True,
                            perf_mode=DR,
                            tile_position=(p0, 0),
                        )

                    for jt in jts:
                        if h == 1 or jt not in ACT_JTS[1 if h == 6 else h % 2]:
                            continue
                        psS = psA_pool.tile([P, N], F32, tag="psA", name="psA")
                        for ih in range(2):
                            smm(psS[:, ih * 512 : (ih + 1) * 512], jt, ih)
                        nc.scalar.activation(
                            P8[h][jt // 2][:, jt % 2, :], psS, EXP, scale=0.125
                        )
                    return flip

                def emit_s_b(h, jts, flip, act_share=False):
                    if h % 4 == 3:
                        Qt, Kt = QsX, KsX
                        p0 = 32 * (h // 4)
                    else:
                        Qt, Kt = Qs[h // 4], Ks[h // 4]
                        p0 = 32 * (h % 4)

                    def smm(out_ap, jt, ih):
                        nc.tensor.matmul(
                            out_ap,
                            lhsT=Kt[p0 : p0 + 32, :, jt * P : (jt + 1) * P],
                            rhs=Qt[p0 : p0 + 32, :, ih * 512 : (ih + 1) * 512],
                            start=True,
                            stop=True,
                            perf_mode=DR,
                            tile_position=(p0, 0),
                        )

                    for jt in jts:
                        if h == 1 or jt in ACT_JTS[1 if h == 6 else h % 2]:
                            continue
                        for ih in range(2):
                            psb = psB_pool.tile([P, 512], F32, tag="psB", name="psB")
                            smm(psb, jt, ih)
                            tgt = P8[h][jt // 2][
                                :, jt % 2, ih * 512 : (ih + 1) * 512
                            ]
                            if act_share and (jt + ih) % 2 == 0:
                                # tail drain: split the halves with the
                                # otherwise-idle ScalarE
                                nc.scalar.activation(
                                    tgt, psb, EXP, scale=0.125
                                )
                            else:
                                nc.vector.tensor_scalar(
                                    tgt.bitcast(I8),
                                    psb,
                                    EXP_C1,
                                    EXP_C2,
                                    op0=AOP.mult,
                                    op1=AOP.add,
                                )
                        flip ^= 1
                    return flip

                def emit_av_norm(pair, ihs=(0, 1)):
                    # pair = (odd head, even head); even head's raw O and
                    # denominator land on partitions 0:64, odd head's on
                    # 64:128, in a shared psO / den tile per i-half.  One
                    # reciprocal (PSUM->SBUF, single-PSUM-input legal) and
                    # one multiply normalize both heads at once.
                    hA, hB = pair
                    cph, gh = hB // 4, (hB % 4) // 2
                    for ih in ihs:
                        sl = slice(ih * 512, (ih + 1) * 512)
                        psO = psO_pool.tile([P, 512], F32, tag="psB", name="psO")
                        den = psO_pool.tile([P, 512], F32, tag="psB", name="den")
                        for k, h in enumerate((hB, hA)):
                            for jp in range(4):
                                nc.tensor.matmul(
                                    psO,
                                    lhsT=VT[jp][:, :, h, :],
                                    rhs=P8[h][jp][:, :, sl],
                                    start=(k == 0 and jp == 0),
                                    stop=(k == 1 and jp == 3),
                                    perf_mode=DR,
                                )
                            for jp in range(4):
                                nc.tensor.matmul(
                                    den,
                                    lhsT=onesAV[:, :, h % 2, :],
                                    rhs=P8[h][jp][:, :, sl],
                                    start=(k == 0 and jp == 0),
                                    stop=(k == 1 and jp == 3),
                                    perf_mode=DR,
                                )
                        rec = rec_pool.tile([P, 512], F32, tag="rec", name="rec")
                        nc.vector.reciprocal(rec, den)
                        nc.vector.tensor_tensor(
                            O8[cph][:, gh, sl], psO, rec, AOP.mult
                        )

                LATE_ENG = [nc.scalar, nc.vector, nc.scalar, nc.vector]

                def late_qk(pos):
                    # deferred Q/K ot 2/3 projections ride the psB/psO slots
                    if pos > 3:
                        return
                    qk, ot = pos % 2, 2 + pos // 2
                    for nh2 in range(2):
                        ps = qk_proj_half(qk, ot, nh2, psB_pool)
                        qk_epi_half(qk, ot, nh2, ps, LATE_ENG[(pos + nh2) % 4])

                # 3-stage pipeline: Act exps for head h, DVE/Pool half-
                # exps for head h-1, AV+normalize for head h-2.  Act's
                # PSUM slots (psA) never queue behind DVE/Pool backlog in
                # the PE's in-order stream.
                def emit_s_first():
                    # head 1 (first emitted) entirely as [128,512] halves:
                    # the ih=0 halves only need the first four projection
                    # epilogues, so exp work starts ~1.5us earlier.
                    Qt, Kt = Qs[0], Ks[0]
                    p0 = 32
                    for ih in range(2):
                        for jt in range(8):
                            psb = psB_pool.tile(
                                [P, 512], F32, tag="psB", name="psB"
                            )
                            nc.tensor.matmul(
                                psb,
                                lhsT=Kt[p0 : p0 + 32, :, jt * P : (jt + 1) * P],
                                rhs=Qt[p0 : p0 + 32, :, ih * 512 : (ih + 1) * 512],
                                start=True,
                                stop=True,
                                perf_mode=DR,
                                tile_position=(p0, 0),
                            )
                            tgt = P8[1][jt // 2][
                                :, jt % 2, ih * 512 : (ih + 1) * 512
                            ]
                            if jt % 2 == 0:
                                nc.scalar.activation(tgt, psb, EXP, scale=0.125)
                            else:
                                nc.vector.tensor_scalar(
                                    tgt.bitcast(I8), psb, EXP_C1, EXP_C2,
                                    op0=AOP.mult, op1=AOP.add,
                                )

                HEAD_ORDER = [1, 0, 3, 2, 5, 4, 7, 6]
                PAIRS = [(1, 0), (3, 2), (5, 4), (7, 6)]
                flip = 0
                for pos, h in enumerate(HEAD_ORDER):
                    if pos == 0:
                        emit_s_first()
                    else:
                        emit_s(h, range(0, 8), flip)
                    if pos == 0:
                        v_proj_all()
                    if pos >= 1:
                        flip = emit_s_b(HEAD_ORDER[pos - 1], range(0, 8), flip)
                    if pos >= 3 and pos % 2 == 1:
                        emit_av_norm(PAIRS[(pos - 3) // 2])
                    late_qk(pos)
                    if pos == 3:
                        nc.sync.dma_start(
                            out=QsX[32:64, :, :], in_=Qs[1][96:P, :, :]
                        )
                        nc.sync.dma_start(
                            out=KsX[32:64, :, :], in_=Ks[1][96:P, :, :]
                        )
                flip = emit_s_b(HEAD_ORDER[7], range(0, 8), flip, act_share=True)
                emit_av_norm(PAIRS[3])

                # ------------- output projection + residual -------------
                # nh2=0 halves are emitted right after the last pair's
                # ih=0 normalize so they overlap its ih=1 tail.
                COPY_ENG = [nc.scalar, nc.vector]

                def po_half(ot, nh2):
                    sl = slice(nh2 * 512, (nh2 + 1) * 512)
                    ob = otile.tile([P, 512], BF16, tag="ob", name="ob")
                    po = psB_pool.tile([P, 512], F32, tag="psB", name=f"po{ot}{nh2}")
                    for cp in range(2):
                        nc.tensor.matmul(
                            po,
                            lhsT=wvo[cp][:, :, 1, ot * P : (ot + 1) * P],
                            rhs=O8[cp][:, :, sl],
                            start=(cp == 0),
                            stop=False,
                            perf_mode=DR,
                        )
                    nc.tensor.matmul(
                        po,
                        lhsT=bo8sb[:, :, ot * P : (ot + 1) * P],
                        rhs=ones8[:, :, :],
                        start=False,
                        stop=False,
                        perf_mode=DR,
                    )
                    nc.tensor.matmul(
                        po,
                        lhsT=ident16,
                        rhs=x16sb[ot][:, sl],
                        start=False,
                        stop=True,
                    )
                    if nh2 == 0:
                        nc.scalar.copy(ob, po)
                    else:
                        nc.vector.tensor_copy(ob, po)
                    # nh0 DMAs on SP; nh1 DMAs on the Act queue, whose
                    # engine work is already finished by the time their
                    # data waits occupy its sequencer.
                    dq = nc.sync if nh2 == 0 else nc.scalar
                    dq.dma_start(out=outd[ot][:, sl], in_=ob)

                for ot in range(4):
                    po_half(ot, 0)
                for ot in range(4):
                    po_half(ot, 1)
    return nc


_BF = ml_dtypes.bfloat16
_F8 = ml_dtypes.float8_e4m3


def _perm_qk():
    # PSUM partition p of Q/K projection tile `ot` holds output row
    # o = (4*(ot//2) + p//32)*64 + 32*(ot%2) + p%32  (head-banded,
    # d-halves split across the DoubleRow group dim).
    j = np.arange(DIM)
    ot, pp = j // P, j % P
    return (4 * (ot // 2) + pp // 32) * HD + 32 * (ot % 2) + (pp % 32)


def _fold8(a):
    # [512, M] c-major -> [ctpair, partition, group, M]
    M = a.shape[1]
    return np.ascontiguousarray(
        a.reshape(2, 2, P, M).transpose(0, 2, 1, 3)
    )


def _prep_maps(x, Wq, bq, Wk, bk, Wv, bv, Wo, bo):
    # plain numpy up front: inputs may arrive as jax device arrays and
    # transforming those would trigger on-device jax execution
    x, Wq, bq, Wk, bk, Wv, bv, Wo, bo = (
        np.asarray(a) for a in (x, Wq, bq, Wk, bk, Wv, bv, Wo, bo)
    )
    B, C, H, W = x.shape
    xf = np.ascontiguousarray(x.reshape(B, C, H * W)).astype(np.float32)
    perm = _perm_qk()
    wq_r = _fold8(Wq.T[:, perm]).astype(_F8)
    wk_r = _fold8(Wk.T[:, perm]).astype(_F8)
    wv_r = _fold8(np.ascontiguousarray(Wv.T)).astype(_F8)
    wo_r = _fold8(np.ascontiguousarray(Wo.T)).astype(_F8)
    bqk = np.concatenate(
        [
            bq[perm].reshape(4, P).T.astype(np.float32),
            bk[perm].reshape(4, P).T.astype(np.float32),
        ],
        axis=1,
    )
    bo8 = np.zeros((1, 2, DIM), _F8)
    bo8[0, 0, :] = bo.astype(_F8)
    shared = {
        "wqk8": np.ascontiguousarray(np.stack([wq_r, wk_r], axis=3)),
        "wvo8": np.ascontiguousarray(np.stack([wv_r, wo_r], axis=3)),
        "bqk": np.ascontiguousarray(bqk),
        "bv": np.asarray(bv, np.float32),
        "bo8": bo8,
        "ones8": np.ones(N, _F8),
        "zeros8": np.zeros(2048, _F8),
        "ident16": np.eye(P, dtype=_BF),
    }
    in_maps = []
    for b in range(B):
        m = dict(shared)
        m["x8"] = np.ascontiguousarray(
            xf[b].reshape(2, 2, P, N).transpose(0, 2, 1, 3)
        ).astype(_F8)
        m["x16"] = xf[b].reshape(4, P, N).astype(_BF)
        in_maps.append(m)
    return in_maps


def kernel(x, Wq, bq, Wk, bk, Wv, bv, Wo, bo, _trace=False):
    from concourse.bass_utils import run_bass_kernel_spmd

    x = np.asarray(x)
    B, C, H, W = x.shape
    in_maps = _prep_maps(x, Wq, bq, Wk, bk, Wv, bv, Wo, bo)
    nc = build_nc()
    res = run_bass_kernel_spmd(nc, in_maps, core_ids=list(range(B)), trace=_trace)
    out = np.stack(
        [np.asarray(res.results[b]["out"]).astype(np.float32) for b in range(B)]
    )
    out = out.reshape(B, C, H, W)
    if _trace:
        kernel.last_results = res
    return out


# revision 61
# speedup vs baseline: 1.0029x; 1.0029x over previous
"""Multi-head attention (dense_transformer) Trainium2 Bass kernel, v2.

Problem: x[8, 512, 32, 32]; per-batch 1x1-conv QKV projections, 8-head
attention over N=H*W=1024 positions (head_dim 64), output projection,
residual. Sharding: data-parallel over batch B=8 across the 8 cores -
one batch element per core, no collectives.

v2 strategy (vs the bf16 baseline): run every matmul in fp8e4 with the
DoubleRow perf mode.  A DoubleRow matmul takes lhsT [K,2,M] / rhs
[K,2,N] and computes sum_g W[:,g].T @ X[:,g] - two 128-deep contraction
steps in one instruction at 0.5 cycles per output row, i.e. 4x the
bf16 matmul throughput per unit of contracted work.  That collapses the
PE time from ~82us to ~30us and shifts the bottleneck to the softmax
exp (~65k free-elems), which is split across three engines:

  - ScalarE (Act): true exp via the activation LUT.
  - VectorE / GpSimd: Schraudolph-style fast exp - one fused
    tensor_scalar (x*log2e + bias) written as int8 IS the fp8e4 bit
    pattern of exp(x/8).  (e4m3 bits b ~= 8*(log2(v)+7), so
    b = s*log2(e) + 56 up to a mantissa correction.)

Other tricks:
  - All five "c = 512" contractions map c -> (ctpair, group, partition)
    so DoubleRow pairs come straight from SBUF layout; Wq/Wk columns are
    host-permuted so each head's Q/K lands as [32, 2, N] (d-halves in
    the group dim) on its own 32-partition band, making S^T a single
    DoubleRow matmul per output tile.
  - V is projected transposed (VT[j, head, d]) with stationary columns
    64..127 set to 1.0, so the AV matmul leaves rows 64..127 of PSUM
    holding the softmax denominator replicated 64x - normalization is
    one partition-offset tensor_tensor divide, no broadcasts.
  - Output projection accumulates Wo.T O (fp8 DoubleRow), + bo (K=1
    DoubleRow bias matmul), + x (bf16 identity matmul residual) in one
    PSUM group; the only epilogue is a PSUM->SBUF copy before DMA out.
"""

import sys

if "/opt/trn_rl_repo" not in sys.path:
    sys.path.insert(0, "/opt/trn_rl_repo")

import numpy as np
import ml_dtypes

import concourse.bass as bass
import concourse.mybir as mybir
from concourse.tile import TileContext

DIM = 512
NH = 8
HD = 64
N = 1024
P = 128
F32 = mybir.dt.float32
BF16 = mybir.dt.bfloat16
F8 = mybir.dt.float8e4
I8 = mybir.dt.int8
AOP = mybir.AluOpType
EXP = mybir.ActivationFunctionType.Exp
DR = mybir.MatmulPerfMode.DoubleRow

# Schraudolph fast-exp constants for e4m3 bits: for logits s (pre 1/8
# scale), bits = s*log2(e) + 7*8 - 0.344 (mantissa-linearity centering)
# + 0.5 (CoreSim truncates on f32->i8 write; HW rounding only shifts
# the softmax by a uniform factor that the normalization cancels).
EXP_C1 = 1.4426950408889634
EXP_C2 = 56.0 - 0.344 + 0.5


class FixedTileContext(TileContext):
    """Works around a walrus/bass snapshot mismatch: this walrus build
    accepts only one sync-wait command per instruction, but Tile's wait
    assigner happily attaches several. After scheduling, excess waits on
    any instruction are peeled off onto same-engine NOPs inserted right
    before it (same blocking semantics: the engine executes in order)."""

    MAX_WAITS = 1
    MAX_WAITS_DATA = 1
    _wsplit_ctr = 0

    def _split_sync_waits(self):
        seq_only = mybir.SEQUENCER_ONLY_OPCODES
        for fn in self.nc.m.functions:
            for blk in fn.blocks:
                insts = list(blk.instructions)
                out = []
                for inst in insts:
                    si = inst.sync_info
                    limit = (
                        self.MAX_WAITS
                        if inst.opcode in seq_only
                        else self.MAX_WAITS_DATA
                    )
                    if si is not None and len(si.on_wait) > limit:
                        waits = list(si.on_wait)
                        movers = waits[:-limit]
                        keep = waits[-limit:]
                        del si.on_wait[:]
                        for w in keep:
                            si.on_wait.append(w)
                        for w in movers:
                            FixedTileContext._wsplit_ctr += 1
                            nop = mybir.InstNoOp(
                                name=f"wsplit-{FixedTileContext._wsplit_ctr}",
                                ins=[],
                                outs=[],
                            )
                            nop.engine = inst.engine
                            nop.sync_info = mybir.SyncInfo(on_wait=[w], on_update=[])
                            out.append(nop)
                    out.append(inst)
                if len(out) != len(insts):
                    del blk.instructions[:]
                    for i in out:
                        blk.add_instruction(i)

    split_on_exit = True

    def __exit__(self, *exc):
        ret = super().__exit__(*exc)
        if exc[0] is None and self.split_on_exit:
            self._split_sync_waits()
        return ret


def build_nc(split_waits=True):
    nc = bass.Bass()

    x8d = nc.dram_tensor("x8", [2, P, 2, N], F8, kind="ExternalInput")
    x16d = nc.dram_tensor("x16", [4, P, N], BF16, kind="ExternalInput")
    wqkd = nc.dram_tensor("wqk8", [2, P, 2, 2, DIM], F8, kind="ExternalInput")
    wvod = nc.dram_tensor("wvo8", [2, P, 2, 2, DIM], F8, kind="ExternalInput")
    bqkd = nc.dram_tensor("bqk", [P, 8], F32, kind="ExternalInput")
    bvd = nc.dram_tensor("bv", [DIM], F32, kind="ExternalInput")
    bo8d = nc.dram_tensor("bo8", [1, 2, DIM], F8, kind="ExternalInput")
    onesd = nc.dram_tensor("ones8", [N], F8, kind="ExternalInput")
    zerosd = nc.dram_tensor("zeros8", [2048], F8, kind="ExternalInput")
    identd = nc.dram_tensor("ident16", [P, P], BF16, kind="ExternalInput")
    outd = nc.dram_tensor("out", [4, P, N], BF16, kind="ExternalOutput")

    FixedTileContext.split_on_exit = split_waits
    with FixedTileContext(nc) as tc:
        with tc.tile_pool(name="persist", bufs=1) as persist, tc.tile_pool(
            name="otile", bufs=4
        ) as otile, tc.tile_pool(name="rec", bufs=2) as rec_pool:
            # ---------------- loads ----------------
            # Startup-critical DMAs are split into halves and spread over
            # the SP and Act HWDGE queues so the first Q-projection group
            # (x8 n-half 0 + the two Wq halves) lands as early as possible.
            x8sb = [
                persist.tile([P, 2, N], F8, tag=f"x8_{cp}", name=f"x8_{cp}")
                for cp in range(2)
            ]
            wqk = [
                persist.tile([P, 2, 2, DIM], F8, tag=f"wqk_{cp}", name=f"wqk_{cp}")
                for cp in range(2)
            ]
            for cp in range(2):
                nc.sync.dma_start(
                    out=x8sb[cp][:, :, 0:512], in_=x8d[cp][:, :, 0:512]
                )
                nc.scalar.dma_start(
                    out=wqk[cp][:, :, 0, :], in_=wqkd[cp][:, :, 0, :]
                )
            bqk_sb = persist.tile([P, 8], F32, tag="bqk", name="bqk")
            nc.sync.dma_start(out=bqk_sb, in_=bqkd[:, :])
            # K weight halves before the x8 i-halves: the shared DMA
            # device serializes transfers in issue order, and the first
            # K-projection matmuls need these ~1.5us before the x8
            # second halves are touched.
            for cp in range(2):
                nc.scalar.dma_start(
                    out=wqk[cp][:, :, 1, :], in_=wqkd[cp][:, :, 1, :]
                )
            for cp in range(2):
                nc.sync.dma_start(
                    out=x8sb[cp][:, :, 512:N], in_=x8d[cp][:, :, 512:N]
                )
            wvo = []
            for cp in range(2):
                t = persist.tile(
                    [P, 2, 2, DIM], F8, tag=f"wvo_{cp}", name=f"wvo_{cp}"
                )
                nc.sync.dma_start(out=t, in_=wvod[cp])
                wvo.append(t)
            # bv broadcast across partitions and the head-group dim via
            # zero strides on the DRAM side.
            bvB = persist.tile([P, 2, NH, HD], F32, tag="bvB", name="bvB")
            bv_ap = bvd[:]
            nc.scalar.dma_start(
                out=bvB,
                in_=bass.AP(
                    tensor=bv_ap.tensor,
                    offset=0,
                    ap=[[0, P], [0, 2], [1, DIM]],
                ),
            )

            # VT tiles: [j-partition, jt-group, head, 128] - V sits in
            # columns 64*(h%2)..+64, the other half is zero, so an M=128
            # matmul at column position 0 (the only legal one) lands the
            # head's raw O on partitions 64*(h%2)..+64 of the pair tile.
            VT = [
                persist.tile([P, 2, NH, P], F8, tag=f"vt_{jp}", name=f"vt_{jp}")
                for jp in range(4)
            ]
            ones_ap = onesd[:]
            zeros_ap = zerosd[:]
            for jp in range(4):
                nc.sync.dma_start(
                    out=VT[jp],
                    in_=bass.AP(
                        tensor=zeros_ap.tensor, offset=0,
                        ap=[[0, P], [1, 2048]],
                    ),
                )
            # denominator stationary operands: ones in the parity half,
            # zeros in the other
            onesAV = persist.tile([P, 2, 2, P], F8, tag="onesAV", name="onesAV")
            nc.sync.dma_start(
                out=onesAV,
                in_=bass.AP(
                    tensor=zeros_ap.tensor, offset=0, ap=[[0, P], [1, 512]]
                ),
            )
            for e in range(2):
                nc.sync.dma_start(
                    out=onesAV[:, :, e, e * HD : (e + 1) * HD],
                    in_=bass.AP(
                        tensor=ones_ap.tensor, offset=0,
                        ap=[[0, P], [1, 2 * HD]],
                    ),
                )
            # phase-3 inputs (SP queue keeps filling while PE works)
            x16sb = []
            for ot in range(4):
                t = persist.tile([P, N], BF16, tag=f"x16_{ot}", name=f"x16_{ot}")
                nc.sync.dma_start(out=t, in_=x16d[ot])
                x16sb.append(t)
            ident16 = persist.tile([P, P], BF16, tag="ident", name="ident")
            nc.sync.dma_start(out=ident16, in_=identd[:, :])
            bo8sb = persist.tile([1, 2, DIM], F8, tag="bo8", name="bo8")
            nc.sync.dma_start(out=bo8sb, in_=bo8d[:, :, :])
            ones8 = persist.tile([1, 2, DIM], F8, tag="ones8", name="ones8")
            nc.sync.dma_start(
                out=ones8,
                in_=bass.AP(tensor=ones_ap.tensor, offset=0, ap=[[0, 1], [1, N]]),
            )

            # persistent attention state
            Qs = [
                persist.tile([P, 2, N], F8, tag=f"qs_{i}", name=f"qs_{i}")
                for i in range(2)
            ]
            Ks = [
                persist.tile([P, 2, N], F8, tag=f"ks_{i}", name=f"ks_{i}")
                for i in range(2)
            ]
            # PE tile row positions only allow 0/32/64 - heads 3 and 7
            # (whose bands sit at partitions 96..127) are DMA-remapped
            # into spare tiles at rows 0:32 / 32:64.
            QsX = persist.tile([P, 2, N], F8, tag="qsx", name="qsx")
            KsX = persist.tile([P, 2, N], F8, tag="ksx", name="ksx")
            P8 = [
                [
                    persist.tile(
                        [P, 2, N], F8, tag=f"p8_{h}_{jp}", name=f"p8_{h}_{jp}"
                    )
                    for jp in range(4)
                ]
                for h in range(NH)
            ]
            O8 = [
                persist.tile([P, 2, N], F8, tag=f"o8_{cp}", name=f"o8_{cp}")
                for cp in range(2)
            ]

            # ------------- Q/K/V projections -------------
            # All phases share one PSUM pool structure (psA: 2x [128,1024]
            # / psB, psO: 2x [128,512]) so no pool-close barrier ever
            # stalls the pipeline between phases.
            ACT_JTS = ((0, 1, 2, 4, 5, 6), (0, 1, 2, 4, 6))

            with tc.tile_pool(
                name="psA", bufs=2, space="PSUM"
            ) as psA_pool, tc.tile_pool(
                name="psB", bufs=4, space="PSUM"
            ) as psB_pool:
                psO_pool = psB_pool

                def qk_proj_half(qk, ot, nh2, pool):
                    ps = pool.tile([P, 512], F32, tag=pool.name, name=f"pp{qk}{ot}{nh2}")
                    for cp in range(2):
                        nc.tensor.matmul(
                            ps,
                            lhsT=wqk[cp][:, :, qk, ot * P : (ot + 1) * P],
                            rhs=x8sb[cp][:, :, nh2 * 512 : (nh2 + 1) * 512],
                            start=(cp == 0),
                            stop=(cp == 1),
                            perf_mode=DR,
                        )
                    return ps

                def qk_epi_half(qk, ot, nh2, ps, eng):
                    tgt = (Qs if qk == 0 else Ks)[ot // 2][
                        :, ot % 2, nh2 * 512 : (nh2 + 1) * 512
                    ]
                    bias = bqk_sb[:, 4 * qk + ot : 4 * qk + ot + 1]
                    if eng is nc.scalar:
                        nc.scalar.activation(
                            tgt, ps, mybir.ActivationFunctionType.Identity, bias=bias
                        )
                    else:
                        eng.tensor_scalar(tgt, ps, bias, None, op0=AOP.add)

                def v_proj_half(vt, g, pool):
                    ps = pool.tile([P, 512], F32, tag=pool.name, name=f"ppv{vt}{g}")
                    jt = 2 * vt + g
                    for cp in range(2):
                        nc.tensor.matmul(
                            ps,
                            lhsT=x8sb[cp][:, :, jt * P : (jt + 1) * P],
                            rhs=wvo[cp][:, :, 0, :],
                            start=(cp == 0),
                            stop=(cp == 1),
                            perf_mode=DR,
                        )
                    return ps

                def v_epi_half(vt, g, ps, eng):
                    vte = VT[vt].rearrange("p g (h2 e) d -> p g h2 e d", e=2)
                    psr = ps.rearrange("p (h2 e d) -> p h2 e d", h2=4, e=2)
                    bvr = bvB.rearrange("p g (h2 e) d -> p g h2 e d", e=2)
                    for e in range(2):
                        eng.tensor_tensor(
                            vte[:, g, :, e, e * HD : (e + 1) * HD],
                            psr[:, :, e, :],
                            bvr[:, g, :, e, :],
                            AOP.add,
                        )

                # Q/K columns 0..1 first (they gate the first head's S);
                # epilogue halves spread over all three engines.  V after.
                for nh2 in range(2):
                    for ot in range(2):
                        for qk in range(2):
                            ps = qk_proj_half(qk, ot, nh2, psB_pool)
                            qk_epi_half(
                                qk, ot, nh2, ps,
                                nc.scalar if qk == 0 else nc.vector,
                            )
                # remap head 3's band to a legal PE row position (head
                # 7's follows once the deferred ot2/3 projections land)
                nc.sync.dma_start(out=QsX[0:32, :, :], in_=Qs[0][96:P, :, :])
                nc.sync.dma_start(out=KsX[0:32, :, :], in_=Ks[0][96:P, :, :])

                V_ENG = [nc.vector] * 8

                def v_proj_all():
                    for vt in range(4):
                        for g in range(2):
                            ps = v_proj_half(vt, g, psB_pool)
                            v_epi_half(vt, g, ps, V_ENG[2 * vt + g])

                # ------------- attention heads -------------
                def emit_s(h, jts, flip):
                    if h % 4 == 3:
                        Qt, Kt = QsX, KsX
                        p0 = 32 * (h // 4)
                    else:
                        Qt, Kt = Qs[h // 4], Ks[h // 4]
                        p0 = 32 * (h % 4)

                    def smm(out_ap, jt, ih):
                        nc.tensor.matmul(
                            out_ap,
                            lhsT=Kt[p0 : p0 + 32, :, jt * P : (jt + 1) * P],
                            rhs=Qt[p0 : p0 + 32, :, ih * 512 : (ih + 1) * 512],
                            start=True,
                            stop=True,
                            perf_mode=DR,
                            tile_position=(p0, 0),
                        )

                    for jt in jts:
                        if h == 1 or jt not in ACT_JTS[1 if h == 6 else h % 2]:
                            continue
                        psS = psA_pool.tile([P, N], F32, tag="psA", name="psA")
                        for ih in range(2):
                            smm(psS[:, ih * 512 : (ih + 1) * 512], jt, ih)
                        nc.scalar.activation(
                            P8[h][jt // 2][:, jt % 2, :], psS, EXP, scale=0.125
                        )
                    return flip

                def emit_s_b(h, jts, flip, act_share=False):
                    if h % 4 == 3:
                        Qt, Kt = QsX, KsX
                        p0 = 32 * (h // 4)
                    else:
                        Qt, Kt = Qs[h // 4], Ks[h // 4]
                        p0 = 32 * (h % 4)

                    def smm(out_ap, jt, ih):
                        nc.tensor.matmul(
                            out_ap,
                            lhsT=Kt[p0 : p0 + 32, :, jt * P : (jt + 1) * P],
                            rhs=Qt[p0 : p0 + 32, :, ih * 512 : (ih + 1) * 512],
                            start=True,
                            stop=True,
                            perf_mode=DR,
                            tile_position=(p0, 0),
                        )

                    for jt in jts:
                        if h == 1 or jt in ACT_JTS[1 if h == 6 else h % 2]:
                            continue
                        for ih in range(2):
                            psb = psB_pool.tile([P, 512], F32, tag="psB", name="psB")
                            smm(psb, jt, ih)
                            tgt = P8[h][jt // 2][
                                :, jt % 2, ih * 512 : (ih + 1) * 512
                            ]
                            if act_share and (jt + ih) % 2 == 0:
                                # tail drain: split the halves with the
                                # otherwise-idle ScalarE
                                nc.scalar.activation(
                                    tgt, psb, EXP, scale=0.125
                                )
                            else:
                                nc.vector.tensor_scalar(
                                    tgt.bitcast(I8),
                                    psb,
                                    EXP_C1,
                                    EXP_C2,
                                    op0=AOP.mult,
                                    op1=AOP.add,
                                )
                        flip ^= 1
                    return flip

                def emit_av_norm(pair, ihs=(0, 1)):
                    # pair = (odd head, even head); even head's raw O and
                    # denominator land on partitions 0:64, odd head's on
                    # 64:128, in a shared psO / den tile per i-half.  One
                    # reciprocal (PSUM->SBUF, single-PSUM-input legal) and
                    # one multiply normalize both heads at once.
                    hA, hB = pair
                    cph, gh = hB // 4, (hB % 4) // 2
                    for ih in ihs:
                        sl = slice(ih * 512, (ih + 1) * 512)
                        psO = psO_pool.tile([P, 512], F32, tag="psB", name="psO")
                        den = psO_pool.tile([P, 512], F32, tag="psB", name="den")
                        for k, h in enumerate((hB, hA)):
                            for jp in range(4):
                                nc.tensor.matmul(
                                    psO,
                                    lhsT=VT[jp][:, :, h, :],
                                    rhs=P8[h][jp][:, :, sl],
                                    start=(k == 0 and jp == 0),
                                    stop=(k == 1 and jp == 3),
                                    perf_mode=DR,
                                )
                            for jp in range(4):
                                nc.tensor.matmul(
                                    den,
                                    lhsT=onesAV[:, :, h % 2, :],
                                    rhs=P8[h][jp][:, :, sl],
                                    start=(k == 0 and jp == 0),
                                    stop=(k == 1 and jp == 3),
                                    perf_mode=DR,
                                )
                        rec = rec_pool.tile([P, 512], F32, tag="rec", name="rec")
                        nc.vector.reciprocal(rec, den)
                        nc.vector.tensor_tensor(
                            O8[cph][:, gh, sl], psO, rec, AOP.mult
                        )

                LATE_ENG = [nc.scalar, nc.vector, nc.scalar, nc.vector]

                def late_qk(pos):
                    # deferred Q/K ot 2/3 projections ride the psB/psO slots
                    if pos > 3:
                        return
                    qk, ot = pos % 2, 2 + pos // 2
                    for nh2 in range(2):
                        ps = qk_proj_half(qk, ot, nh2, psB_pool)
                        qk_epi_half(qk, ot, nh2, ps, LATE_ENG[(pos + nh2) % 4])

                # 3-stage pipeline: Act exps for head h, DVE/Pool half-
                # exps for head h-1, AV+normalize for head h-2.  Act's
                # PSUM slots (psA) never queue behind DVE/Pool backlog in
                # the PE's in-order stream.
                def emit_s_first():
                    # head 1 (first emitted) entirely as [128,512] halves:
                    # the ih=0 halves only need the first four projection
                    # epilogues, so exp work starts ~1.5us earlier.
                    Qt, Kt = Qs[0], Ks[0]
                    p0 = 32
                    for ih in range(2):
                        for jt in range(8):
                            psb = psB_pool.tile(
                                [P, 512], F32, tag="psB", name="psB"
                            )
                            nc.tensor.matmul(
                                psb,
                                lhsT=Kt[p0 : p0 + 32, :, jt * P : (jt + 1) * P],
                                rhs=Qt[p0 : p0 + 32, :, ih * 512 : (ih + 1) * 512],
                                start=True,
                                stop=True,
                                perf_mode=DR,
                                tile_position=(p0, 0),
                            )
                            tgt = P8[1][jt // 2][
                                :, jt % 2, ih * 512 : (ih + 1) * 512
                            ]
                            if jt % 2 == 0:
                                nc.scalar.activation(tgt, psb, EXP, scale=0.125)
                            else:
                                nc.vector.tensor_scalar(
                                    tgt.bitcast(I8), psb, EXP_C1, EXP_C2,
                                    op0=AOP.mult, op1=AOP.add,
                                )

                HEAD_ORDER = [1, 0, 3, 2, 5, 4, 7, 6]
                PAIRS = [(1, 0), (3, 2), (5, 4), (7, 6)]
                flip = 0
                for pos, h in enumerate(HEAD_ORDER):
                    if pos == 0:
                        emit_s_first()
                    else:
                        emit_s(h, range(0, 8), flip)
                    if pos == 0:
                        v_proj_all()
                    if pos >= 1:
                        flip = emit_s_b(HEAD_ORDER[pos - 1], range(0, 8), flip)
                    if pos >= 3 and pos % 2 == 1:
                        emit_av_norm(PAIRS[(pos - 3) // 2])
                    late_qk(pos)
                    if pos == 3:
                        nc.sync.dma_start(
                            out=QsX[32:64, :, :], in_=Qs[1][96:P, :, :]
                        )
                        nc.sync.dma_start(
                            out=KsX[32:64, :, :], in_=Ks[1][96:P, :, :]
                        )
                flip = emit_s_b(HEAD_ORDER[7], range(0, 8), flip, act_share=True)
                emit_av_norm(PAIRS[3])

                # ------------- output projection + residual -------------
                # nh2=0 halves are emitted right after the last pair's
                # ih=0 normalize so they overlap its ih=1 tail.
                COPY_ENG = [nc.scalar, nc.vector]

                def po_half(ot, nh2):
                    sl = slice(nh2 * 512, (nh2 + 1) * 512)
                    ob = otile.tile([P, 512], BF16, tag="ob", name="ob")
                    po = psB_pool.tile([P, 512], F32, tag="psB", name=f"po{ot}{nh2}")
                    for cp in range(2):
                        nc.tensor.matmul(
                            po,
                            lhsT=wvo[cp][:, :, 1, ot * P : (ot + 1) * P],
                            rhs=O8[cp][:, :, sl],
                            start=(cp == 0),
                            stop=False,
                            perf_mode=DR,
                        )
                    nc.tensor.matmul(
                        po,
                        lhsT=bo8sb[:, :, ot * P : (ot + 1) * P],
                        rhs=ones8[:, :, :],
                        start=False,
                        stop=False,
                        perf_mode=DR,
                    )
                    nc.tensor.matmul(
                        po,
                        lhsT=ident16,
                        rhs=x16sb[ot][:, sl],
                        start=False,
                        stop=True,
                    )
                    if nh2 == 0:
                        nc.scalar.copy(ob, po)
                    else:
                        nc.vector.tensor_copy(ob, po)
                    # out-DMAs all ride the SP queue so their data waits
                    # never block an engine's sequencer.
                    nc.sync.dma_start(out=outd[ot][:, sl], in_=ob)

                for ot in range(4):
                    po_half(ot, 0)
                for ot in range(4):
                    po_half(ot, 1)
    return nc


_BF = ml_dtypes.bfloat16
_F8 = ml_dtypes.float8_e4m3


def _perm_qk():
    # PSUM partition p of Q/K projection tile `ot` holds output row
    # o = (4*(ot//2) + p//32)*64 + 32*(ot%2) + p%32  (head-banded,
    # d-halves split across the DoubleRow group dim).
    j = np.arange(DIM)
    ot, pp = j // P, j % P
    return (4 * (ot // 2) + pp // 32) * HD + 32 * (ot % 2) + (pp % 32)


def _fold8(a):
    # [512, M] c-major -> [ctpair, partition, group, M]
    M = a.shape[1]
    return np.ascontiguousarray(
        a.reshape(2, 2, P, M).transpose(0, 2, 1, 3)
    )


def _prep_maps(x, Wq, bq, Wk, bk, Wv, bv, Wo, bo):
    # plain numpy up front: inputs may arrive as jax device arrays and
    # transforming those would trigger on-device jax execution
    x, Wq, bq, Wk, bk, Wv, bv, Wo, bo = (
        np.asarray(a) for a in (x, Wq, bq, Wk, bk, Wv, bv, Wo, bo)
    )
    B, C, H, W = x.shape
    xf = np.ascontiguousarray(x.reshape(B, C, H * W)).astype(np.float32)
    perm = _perm_qk()
    wq_r = _fold8(Wq.T[:, perm]).astype(_F8)
    wk_r = _fold8(Wk.T[:, perm]).astype(_F8)
    wv_r = _fold8(np.ascontiguousarray(Wv.T)).astype(_F8)
    wo_r = _fold8(np.ascontiguousarray(Wo.T)).astype(_F8)
    bqk = np.concatenate(
        [
            bq[perm].reshape(4, P).T.astype(np.float32),
            bk[perm].reshape(4, P).T.astype(np.float32),
        ],
        axis=1,
    )
    bo8 = np.zeros((1, 2, DIM), _F8)
    bo8[0, 0, :] = bo.astype(_F8)
    shared = {
        "wqk8": np.ascontiguousarray(np.stack([wq_r, wk_r], axis=3)),
        "wvo8": np.ascontiguousarray(np.stack([wv_r, wo_r], axis=3)),
        "bqk": np.ascontiguousarray(bqk),
        "bv": np.asarray(bv, np.float32),
        "bo8": bo8,
        "ones8": np.ones(N, _F8),
        "zeros8": np.zeros(2048, _F8),
        "ident16": np.eye(P, dtype=_BF),
    }
    in_maps = []
    for b in range(B):
        m = dict(shared)
        m["x8"] = np.ascontiguousarray(
            xf[b].reshape(2, 2, P, N).transpose(0, 2, 1, 3)
        ).astype(_F8)
        m["x16"] = xf[b].reshape(4, P, N).astype(_BF)
        in_maps.append(m)
    return in_maps


def kernel(x, Wq, bq, Wk, bk, Wv, bv, Wo, bo, _trace=False):
    from concourse.bass_utils import run_bass_kernel_spmd

    x = np.asarray(x)
    B, C, H, W = x.shape
    in_maps = _prep_maps(x, Wq, bq, Wk, bk, Wv, bv, Wo, bo)
    nc = build_nc()
    res = run_bass_kernel_spmd(nc, in_maps, core_ids=list(range(B)), trace=_trace)
    out = np.stack(
        [np.asarray(res.results[b]["out"]).astype(np.float32) for b in range(B)]
    )
    out = out.reshape(B, C, H, W)
    if _trace:
        kernel.last_results = res
    return out
